# revision 34
# baseline (speedup 1.0000x reference)
"""Trainium2 Bass kernel for windowed Conv1d(k=3) + sigmoid gating.

Reference (B=16, T=960, D=1024, W=10): windows of 10 conv'd independently
with per-window zero pad 1:
    cnn[t, d] = sum_{k,c} conv_w[d, c, k] * xpad[t + k, c]
    out = cnn * sigmoid(cnn @ gate_w.T + gate_b)

Strategy: data parallel over 8 cores (2 batches / 192 windows / core).
The conv uses hybrid Winograd: two F(4,3) tiles (outputs 0-3 from
xp[0:6], outputs 4-7 from xp[4:10]) at points {0,1,-1,2,-1/2,inf} plus an
F(2,3) tail (outputs 8,9 from xp[8:12]) whose 4 products FOLD into the
same 6 weight matrices by choosing the tail's interpolation points as a
subset of the F43 points.  Rotating the tail point-set over 3 window
classes (w%3) equalizes the streams:
    T1=(0,1,-1,inf)->streams(0,1,2,5)  T2=(0,2,-1/2,inf)->(0,3,4,5)
    T3=(1,-1,2,-1/2)->(1,2,3,4)
so every stream carries exactly 8 columns per 3 windows: 16 muls/window
(vs 30 direct, 20 for F(2,3)) => conv PE time ~82us + gate ~52us.

Streams use canonical Vandermonde weights g_b = [1,b,b^2] . W (the
normalization lives in the host-side input transforms V^{-T}).  Per core:
2 groups of 96 windows; per (group, dck): 6 accumulation chains of N=256
(PSUM: 3 banks/slot, 2 slots + 2 gate banks = 8).  The A^T combine runs
on ScalarE/VectorE/GpSimd under the matmul stream with bf16 intermediates.
Host does padding, transposition to channel-major, input transforms, and
weight transforms (f64 -> bf16).
"""

import numpy as np
import ml_dtypes

import concourse.bacc as bacc
import concourse.bass as bass
import concourse.tile as tile
from concourse import mybir
from concourse.bass_utils import run_bass_kernel_spmd

BF16 = ml_dtypes.bfloat16

B, T, D, W = 16, 960, 1024, 10
NCORES = 8
BC = B // NCORES             # batches per core (2)
NWIN = BC * T // W           # windows per core (192)
RC = NWIN * W                # output rows per core (1920)
NG = 2                       # groups per core
GWN = NWIN // NG             # windows per group (96)
GN = GWN * W                 # output cols per group (960)
NS = 6                       # winograd streams
SCOL = GWN * 16 // 6         # stream cols per group (256)
NCH = D // 128               # d chunks (8)
NT = GWN // 3                # tail windows per class per group (32)
AF = mybir.ActivationFunctionType

INF = "inf"
PTS = [0.0, 1.0, -1.0, 2.0, -0.5, INF]
T1_PTS = [0.0, 1.0, -1.0, INF]; T1_MAP = [0, 1, 2, 5]
T2_PTS = [0.0, 2.0, -0.5, INF]; T2_MAP = [0, 3, 4, 5]
T3_PTS = [1.0, -1.0, 2.0, -0.5]; T3_MAP = [1, 2, 3, 4]
# class feeding stream j at slot X / Y (class index 0/1/2 = T1/T2/T3)
XY = {0: (0, 1), 1: (0, 2), 2: (0, 2), 3: (1, 2), 4: (1, 2), 5: (0, 1)}
TAIL = [(T1_PTS, T1_MAP), (T2_PTS, T2_MAP), (T3_PTS, T3_MAP)]


def _vinv_T(points):
    n = len(points)
    V = np.zeros((n, n))
    for j, b in enumerate(points):
        if b is INF:
            V[j, n - 1] = 1.0
        else:
            V[j] = [b ** i for i in range(n)]
    return np.linalg.inv(V).T


BA = _vinv_T(PTS)
BT = [_vinv_T(p) for p, _ in TAIL]


# storage order of streams (chains emitted 0..5 in storage order): logical
# stream L[s]; m1/m2 first so the combine can start mid-unit
SJ = [1, 2, 3, 4, 0, 5]
SP = {j: s for s, j in enumerate(SJ)}   # logical -> storage


def _build():
    nc = bacc.Bacc("TRN2", target_bir_lowering=False, debug=False)

    # xt[g*2+h]: [cc, (s_local, ck, col)] transformed input, storage-stream
    # triples per half -> 12KB rows per DMA
    xt = nc.dram_tensor("xt", [NG * 2, 128, 3 * NCH * SCOL], mybir.dt.bfloat16,
                        kind="ExternalInput")
    # cwr[dck]: [cc, ((j*NCH+ck)*128 + dd)] conv lhsT blocks
    cwr = nc.dram_tensor("cwr", [NCH, 128, NS * NCH * 128], mybir.dt.bfloat16,
                         kind="ExternalInput")
    # gwr[eck]: [dd, (dck*128 + ee)] gate lhsT blocks
    gwr = nc.dram_tensor("gwr", [NCH, 128, NCH * 128], mybir.dt.bfloat16,
                         kind="ExternalInput")
    cb = nc.dram_tensor("cb", [128, NCH], mybir.dt.float32, kind="ExternalInput")
    gb = nc.dram_tensor("gb", [128, NCH], mybir.dt.float32, kind="ExternalInput")
    outT = nc.dram_tensor("outT", [D, RC], mybir.dt.float32, kind="ExternalOutput")

    with tile.TileContext(nc) as tc:
        with (
            tc.tile_pool(name="consts", bufs=1) as consts,
            tc.tile_pool(name="xtp", bufs=1) as xtp,
            tc.tile_pool(name="cnn", bufs=1) as cnnp,
            tc.tile_pool(name="work", bufs=3) as work,
            tc.tile_pool(name="gout", bufs=4) as gout,
            tc.tile_pool(name="cpsum", bufs=2, space="PSUM") as cpsum,
            tc.tile_pool(name="gpsum", bufs=2, space="PSUM") as gpsum,
        ):
            cwr_sb = [None] * NCH
            xt_sb = [None, None]        # per half

            def load_xt_half(g, h):
                t = xtp.tile([128, 3 * NCH * SCOL], mybir.dt.bfloat16,
                             tag=f"xt{h}")
                nc.sync.dma_start(t[:], xt[g * 2 + h])
                xt_sb[h] = t

            def cw_tile(dck):
                cwt = consts.tile([128, NS * NCH * 128],
                                  mybir.dt.bfloat16, tag=f"cw{dck}")
                cwr_sb[dck] = cwt
                return cwt

            # ramp: half-group interleave in first-use order
            cw0 = cw_tile(0)
            HB = 3 * NCH * 128
            load_xt_half(0, 0)
            nc.sync.dma_start(cw0[:, :HB], cwr[0][:, :HB])
            load_xt_half(0, 1)
            nc.sync.dma_start(cw0[:, HB:], cwr[0][:, HB:])
            cb_sb = consts.tile([128, NCH], mybir.dt.float32, tag="cb")
            nc.sync.dma_start(cb_sb[:], cb[:])
            gb_sb = consts.tile([128, NCH], mybir.dt.float32, tag="gb")
            nc.sync.dma_start(gb_sb[:], gb[:])
            for dck in range(1, NCH):
                nc.sync.dma_start(cw_tile(dck)[:], cwr[dck])
            gwr_sb = []
            for eck in range(NCH):
                t = consts.tile([128, NCH * 128], mybir.dt.bfloat16, tag=f"gw{eck}")
                nc.sync.dma_start(t[:], gwr[eck])
                gwr_sb.append(t)

            # HAM warmup during the DMA ramp
            scr = consts.tile([128, 512], mybir.dt.bfloat16, tag="scr")
            nc.gpsimd.memset(scr[:], 0.0)
            for _ in range(24):
                wps = gpsum.tile([128, 480], mybir.dt.float32, tag="gps")
                nc.tensor.matmul(wps[:], scr[:, :128], scr[:, :480],
                                 start=True, stop=True)

            cnn_t = [[None] * NCH for _ in range(NG)]

            def conv_unit(g, dck, light=False):
                ps = cpsum.tile([128, NS * SCOL], mybir.dt.float32, tag="cps")
                for s in range(NS):
                    h, sl = s // 3, s % 3
                    for ck in range(NCH):
                        nc.tensor.matmul(
                            ps[:, s * SCOL:(s + 1) * SCOL],
                            cwr_sb[dck][:, (s * NCH + ck) * 128:
                                        (s * NCH + ck + 1) * 128],
                            xt_sb[h][:, (sl * NCH + ck) * SCOL:
                                     (sl * NCH + ck + 1) * SCOL],
                            start=(ck == 0),
                            stop=(ck == NCH - 1),
                        )
                combine(g, dck, ps, light)

            def combine(g, dck, ps, light=False):
                AB = 2 * GWN                     # 192
                XL, XH = AB, AB + NT             # X block 192:224
                YH = XH + NT                     # Y block 224:256
                cbs = cb_sb[:, dck:dck + 1]
                bf = mybir.dt.bfloat16

                def m(j, lo=0, hi=SCOL):
                    s = SP[j]
                    return ps[:, s * SCOL + lo:s * SCOL + hi]

                def st(tag, n=SCOL):
                    return work.tile([128, n], bf, tag=tag, name=tag)

                # cnn cols = t*96 + w' (t-major; w' = cls*32 + w//3) so every
                # combine write below is contiguous in the free dim
                cnn = cnnp.tile([128, GN], bf, tag=f"cnn{g}_{dck}")
                cnn_t[g][dck] = cnn
                v = cnn[:].rearrange("p (t w) -> p t w", w=GWN)

                def outab(t0):
                    # A-tile output t0 and B-tile output t0+4: [128, 2, 96]
                    return v[:, t0:t0 + 5:4]

                # PSUM-reading ops all run early (no slow deps) so the
                # slot frees right after the unit's last chain; SBUF-only
                # tail ops may lag a unit behind without stalling the PE.
                c2s = st("c2s"); nc.scalar.activation(c2s[:], m(2), AF.Copy)
                s1 = st("s1"); nc.vector.tensor_add(s1[:], m(1), c2s[:])
                d1 = st("d1"); nc.vector.tensor_sub(d1[:], m(1), c2s[:])
                c4s = st("c4s"); nc.scalar.activation(c4s[:], m(4), AF.Copy)
                # near phase ends (`light`) keep ScalarE free for the next
                # gates' sigmoids by running the scaled copies on DVE
                def usc(t, src_ap, sc):
                    if light:
                        nc.vector.tensor_scalar(t, src_ap, sc, cbs,
                                                mybir.AluOpType.mult,
                                                mybir.AluOpType.add)
                    else:
                        nc.scalar.activation(t, src_ap, AF.Identity,
                                             bias=cbs, scale=sc)

                def hsc(t, src_ap, sc):
                    if light:
                        nc.vector.tensor_scalar_mul(t, src_ap, sc)
                    else:
                        nc.scalar.activation(t, src_ap, AF.Identity, scale=sc)

                u1 = st("u1"); usc(u1[:], m(3), 2.0)
                h1 = st("h1"); hsc(h1[:], m(4), -0.5)
                u2 = st("u2", AB); usc(u2[:], m(3, 0, AB), 4.0)
                h2 = st("h2", AB); hsc(h2[:], m(4, 0, AB), 0.25)
                u3 = st("u3", AB); usc(u3[:], m(3, 0, AB), 8.0)
                h3 = st("h3", AB); hsc(h3[:], m(4, 0, AB), -0.125)
                a0 = st("a0"); nc.vector.tensor_add(a0[:], m(3), c4s[:])
                t0 = st("t0"); nc.vector.tensor_scalar_add(t0[:], a0[:], cbs)
                P = st("P", XH)
                nc.vector.tensor_add(P[:], m(0, 0, XH), s1[:, :XH])
                nc.vector.tensor_add(v[:, 8, NT:2 * NT], m(0, XH, YH),
                                     t0[:, XL:XH])                        # y8 T2
                c5s = st("c5s")
                nc.vector.tensor_scalar_add(c5s[:], m(5), 0.0)
                # ---- SBUF-only from here on ----
                w1 = st("w1"); nc.gpsimd.tensor_add(w1[:], d1[:], u1[:])
                w2 = st("w2", AB); nc.gpsimd.tensor_add(w2[:], s1[:, :AB], u2[:])
                w3 = st("w3", AB); nc.gpsimd.tensor_add(w3[:], d1[:, :AB], u3[:])
                x3 = st("x3", AB); nc.gpsimd.tensor_add(x3[:], w3[:], h3[:])
                nc.gpsimd.tensor_add(outab(0), P[:, :AB], t0[:, :AB])     # y0
                nc.gpsimd.tensor_add(outab(1), w1[:, :AB], h1[:, :AB])    # y1
                nc.gpsimd.tensor_add(outab(2), w2[:], h2[:])              # y2
                nc.vector.tensor_add(outab(3), x3[:], c5s[:, :AB])        # y3
                # tails: T1 -> w' 0:32, T2 -> 32:64, T3 -> 64:96
                nc.vector.tensor_scalar_add(v[:, 8, 0:NT], P[:, XL:XH], cbs)
                e1 = st("e1", NT)
                nc.vector.tensor_add(e1[:], d1[:, XL:XH], c5s[:, XL:XH])
                nc.vector.tensor_scalar_add(v[:, 9, 0:NT], e1[:], cbs)    # y9 T1
                v2 = st("v2", NT)
                nc.vector.tensor_add(v2[:], u1[:, XL:XH], h1[:, XL:XH])
                nc.vector.tensor_add(v[:, 9, NT:2 * NT], v2[:], c5s[:, XH:YH])
                nc.gpsimd.tensor_add(v[:, 8, 2 * NT:], s1[:, XH:YH],
                                     t0[:, XH:YH])                        # y8 T3
                nc.gpsimd.tensor_add(v[:, 9, 2 * NT:], w1[:, XH:YH],
                                     h1[:, XH:YH])                        # y9 T3

            def gate_unit(g, eck, last=False):
                for c in range(2):
                    ps2 = gpsum.tile([128, 480], mybir.dt.float32, tag="gps")
                    for dck in range(NCH):
                        nc.tensor.matmul(
                            ps2[:],
                            gwr_sb[eck][:, dck * 128:(dck + 1) * 128],
                            cnn_t[g][dck][:, c * 480:(c + 1) * 480],
                            start=(dck == 0),
                            stop=(dck == NCH - 1),
                        )
                    gt = gout.tile([128, 480], mybir.dt.bfloat16, tag="gate")
                    ot = gout.tile([128, 480], mybir.dt.float32, tag="out")
                    chunks = ((0, 240), (240, 480)) if (last and c == 1) \
                        else ((0, 480),)
                    for lo, hi in chunks:
                        nc.scalar.activation(gt[:, lo:hi], ps2[:, lo:hi],
                                             AF.Sigmoid,
                                             bias=gb_sb[:, eck:eck + 1])
                        nc.vector.tensor_mul(ot[:, lo:hi],
                                             cnn_t[g][eck][:, c * 480 + lo:
                                                           c * 480 + hi],
                                             gt[:, lo:hi])
                        nc.sync.dma_start(
                            outT[eck * 128:(eck + 1) * 128,
                                 g * GN + c * 480 + lo:g * GN + c * 480 + hi],
                            ot[:, lo:hi])

            # phase g0 convs (cwr streams behind; xt g0 in half-chunks);
            # after the early units the PE waits on the cwr stream -- pad
            # the known gaps with throwaway matmuls so HAM stays at 8/8
            for dck in range(NCH):
                conv_unit(0, dck, light=(dck >= NCH - 2))
                if dck < 4:
                    wps = gpsum.tile([128, 480], mybir.dt.float32, tag="gps")
                    nmm = 12 if dck < 3 else 8
                    for _ in range(nmm):
                        nc.tensor.matmul(wps[:, :256], scr[:, :128],
                                         scr[:, :256], start=True, stop=True)
            # g0 gates eck 0..2 cover the xt g1 reload window
            gate_unit(0, 0)
            load_xt_half(1, 0)
            load_xt_half(1, 1)
            gate_unit(0, 1)
            gate_unit(0, 2)
            # phase g1 convs
            for dck in range(NCH):
                conv_unit(1, dck, light=(dck >= NCH - 2))
            # remaining g0 gates cover the final combines' engine backlog
            for eck in range(3, NCH):
                gate_unit(0, eck)
            for eck in range(NCH):
                gate_unit(1, eck, last=(eck == NCH - 1))
    nc.compile()
    return nc


def _stream_weights(conv_w):
    W0, W1, W2 = [conv_w[:, :, k].astype(np.float64) for k in range(3)]
    g = []
    for b in PTS:
        g.append(W2 if b is INF else W0 + b * W1 + b * b * W2)
    return np.stack(g)  # [6, Dout, Din]


def _prep_weights(conv_w, conv_b, gate_w, gate_b):
    garr = _stream_weights(conv_w)[SJ]                   # [6 storage, Do, Di]
    # cwr[dck][cc, (s*8+ck)*128+dd] = g_{SJ[s]}[dck*128+dd, ck*128+cc]
    gv = garr.reshape(NS, NCH, 128, NCH, 128)            # [s, dck, dd, ck, cc]
    cw_host = np.ascontiguousarray(gv.transpose(1, 4, 0, 3, 2)).reshape(
        NCH, 128, NS * NCH * 128).astype(BF16)
    gwt = gate_w.T.reshape(NCH, 128, NCH, 128)           # [dck, dd, eck, ee]
    gw_host = np.ascontiguousarray(gwt.transpose(2, 1, 0, 3)).reshape(
        NCH, 128, NCH * 128).astype(BF16)
    cb_host = np.ascontiguousarray(conv_b.reshape(NCH, 128).T).astype(np.float32)
    gb_host = np.ascontiguousarray(gate_b.reshape(NCH, 128).T).astype(np.float32)
    return cw_host, gw_host, cb_host, gb_host


def _prep_core_x(x_shard):
    # x_shard [BC, T, D] -> xt [NG*2, 128, 3*NCH*SCOL] (storage-order halves)
    xw = x_shard.reshape(NWIN, W, D).astype(np.float64)
    xp = np.pad(xw, ((0, 0), (1, 1), (0, 0)))            # [192, 12, D]
    xt_host = np.empty((NG * 2, 128, 3 * NCH * SCOL), BF16)
    # class-major window order within a group: w' = cls*32 + w//3
    perm = np.array([3 * wi + cls for cls in range(3) for wi in range(NT)])
    for g in range(NG):
        ws = xp[g * GWN:(g + 1) * GWN]
        xA = np.einsum('ji,wic->jwc', BA, ws[perm][:, 0:6])  # [6, 96, D]
        xB = np.einsum('ji,wic->jwc', BA, ws[perm][:, 4:10])
        xTl = []
        for ci, (pts, mp) in enumerate(TAIL):
            wc = ws[ci::3][:, 8:12]                      # [32, 4, D]
            xTl.append(np.einsum('ji,wic->jwc', BT[ci], wc))
        for s in range(NS):
            j = SJ[s]
            cX, cY = XY[j]
            jX = TAIL[cX][1].index(j)
            jY = TAIL[cY][1].index(j)
            S = np.concatenate([xA[j], xB[j], xTl[cX][jX], xTl[cY][jY]],
                               axis=0)                   # [256, D]
            blk = S.T.reshape(NCH, 128, SCOL).transpose(1, 0, 2)
            h, sl = s // 3, s % 3
            xt_host[g * 2 + h, :, sl * NCH * SCOL:(sl + 1) * NCH * SCOL] = \
                blk.reshape(128, NCH * SCOL).astype(BF16)
    return xt_host


def _unshard_core(o):
    # o: [D, RC] cols ordered (g, t, cls, widx); window w = 3*widx + cls
    return np.ascontiguousarray(
        o.reshape(D, NG, W, 3, NT).transpose(1, 4, 3, 2, 0).reshape(BC, T, D))


_NC_CACHE = None


def _prep_in_maps(x, conv_w, conv_b, gate_w, gate_b):
    cw_host, gw_host, cb_host, gb_host = _prep_weights(
        conv_w, conv_b, gate_w, gate_b)
    return [
        {"xt": _prep_core_x(x[BC * i:BC * (i + 1)]), "cwr": cw_host,
         "gwr": gw_host, "cb": cb_host, "gb": gb_host}
        for i in range(NCORES)
    ]


def kernel(x, conv_w, conv_b, gate_w, gate_b):
    global _NC_CACHE
    x = np.asarray(x, np.float32)
    conv_w = np.asarray(conv_w, np.float32)
    conv_b = np.asarray(conv_b, np.float32)
    gate_w = np.asarray(gate_w, np.float32)
    gate_b = np.asarray(gate_b, np.float32)

    in_maps = _prep_in_maps(x, conv_w, conv_b, gate_w, gate_b)
    if _NC_CACHE is None:
        _NC_CACHE = _build()
    res = run_bass_kernel_spmd(_NC_CACHE, in_maps,
                               core_ids=list(range(NCORES))).results

    out = np.empty((B, T, D), np.float32)
    for i in range(NCORES):
        out[BC * i:BC * (i + 1)] = _unshard_core(np.asarray(res[i]["outT"]))
    return out
